# revision 20
# baseline (speedup 1.0000x reference)
"""DiT dual-stream attention (B=4, S=2048, D=1024, H=16, DK=DV=64) on 8 TRN2 cores.

Sharding: core i handles batch b = i//2 and head-group g = i%2 (8 heads each).

v3: fully SBUF-resident k/v/q (bf16), no DRAM round-trip for activations.
Phases: A1 = K-projection sweep; A2 = V-projection + slab-0 Q-projection +
score/exp warmup; then a software-pipelined attention loop over 32 head
blocks where scores(G), AV(G-1), Q-proj(next slab) and Y-proj(prev slab)
interleave in the PE queue (keeps the tensor engine continuously busy so it
holds the 2.4GHz pstate) while the scalar engine streams the exp()s (the
~370us bottleneck). Softmax denominator: DVE f32 add-tree over bf16 exp
tiles + ones-matmul broadcast + reciprocal_approx_fast. Bias-add + output
DMA + pairwise AllReduce per 512-token slab on the Pool engine, overlapped.
"""

import os
import sys

for _p in ("/opt/trn_rl_repo", "/root/.axon_site/_ro/trn_rl_repo"):
    if os.path.isdir(_p) and _p not in sys.path:
        sys.path.insert(0, _p)

import numpy as np

import concourse.bass as bass
import concourse.tile as tile
from concourse import bacc, mybir


F32 = mybir.dt.float32
F32R = mybir.dt.float32r
BF16 = mybir.dt.bfloat16

N_CORES = 8
B, S, D = 4, 2048, 1024
H, DK, DV = 16, 64, 64
HL = 8            # local heads per core
FQK = HL * DK     # 512: local q/k width per stream (x or c)
NJ = D // 128     # 8 contraction d-tiles
NP = 4            # token slabs of 512
PAN = S // NP     # 512
NT = S // 128     # 16 k-token tiles
CK = 256          # chunk tokens for projections
NCK = S // CK     # 8 chunks per sweep
SCALE = 1.0 / np.sqrt(np.float32(DK))
EXP_BUFS = 18


def _build_nc():
    nc = bacc.Bacc("TRN2", num_devices=N_CORES)

    xt_in = nc.dram_tensor("xt", [D, S], F32, kind="ExternalInput")
    ct_in = nc.dram_tensor("ct", [D, S], F32, kind="ExternalInput")
    w_names = ["wqx", "wqc", "wkx", "wkc", "wvx", "wvc"]
    w_in = {n: nc.dram_tensor(n, [D, FQK], F32, kind="ExternalInput") for n in w_names}
    wp_in = nc.dram_tensor("wp", [HL * 2 * DV, D], F32, kind="ExternalInput")
    bias_in = nc.dram_tensor("bias", [D], F32, kind="ExternalInput")
    y_out = nc.dram_tensor("y", [S, D], F32, kind="ExternalOutput")

    y_part = [nc.dram_tensor(f"y_part{i}", [PAN, D], F32) for i in range(NP)]
    y_red = [nc.dram_tensor(f"y_red{i}", [PAN, D], F32) for i in range(NP)]

    xr = xt_in.rearrange("(j p) t -> p j t", p=128)
    cr = ct_in.rearrange("(j p) t -> p j t", p=128)

    with tile.TileContext(nc) as tc:
        with (
            tc.tile_pool(name="consts", bufs=1) as consts,
            tc.tile_pool(name="kres", bufs=1) as kres,
            tc.tile_pool(name="vres", bufs=1) as vres,
            tc.tile_pool(name="qres", bufs=2) as qres,
            tc.tile_pool(name="wqp", bufs=1) as wqp,
            tc.tile_pool(name="expool", bufs=EXP_BUFS) as expool,
        ):
            ones_f = consts.tile([128, 128], F32)
            nc.vector.memset(ones_f, 1.0)
            ones_r = consts.tile([128, 128], F32R)
            nc.vector.tensor_copy(ones_r, ones_f)

            k_sb = kres.tile([128, HL, S], BF16, tag="k_sb")
            k_re = k_sb.rearrange("p (fi hp) t -> p fi hp t", fi=4)
            v_sb = vres.tile([128, NT, HL, 128], BF16, tag="v_sb")
            # q slab ring: 2 bufs
            q_tiles = [qres.tile([128, HL, PAN], BF16, tag="q_sb",
                                 name=f"q_sb{i}") for i in range(2)]
            wq_b = {}
            for stream in ("x", "c"):
                wq_b[stream] = wqp.tile([128, NJ, FQK], BF16, tag=f"wq{stream}",
                                        name=f"wq_{stream}")

            def load_cast_w(name, dst, wland_pool, eng):
                land = wland_pool.tile([128, NJ, FQK], F32, tag="wland",
                                       name=f"land_{name}")
                nc.sync.dma_start(
                    out=land, in_=w_in[name].rearrange("(j p) f -> p j f", p=128)
                )
                eng.tensor_copy(dst, land)

            def load_chunk(landp, xcbp, src_r, ck, tag):
                land = landp.tile([128, NJ, CK], F32, tag="land", name=f"ld_{tag}")
                nc.sync.dma_start(out=land, in_=src_r[:, :, ck * CK:(ck + 1) * CK])
                xcb = xcbp.tile([128, NJ, CK], BF16, tag="xcb", name=f"cb_{tag}")
                nc.gpsimd.tensor_copy(xcb, land)
                return xcb

            # ---------------- phase A1: K projection sweep ----------------
            with (
                tc.tile_pool(name="a1w", bufs=2) as a1w,
                tc.tile_pool(name="a1wb", bufs=2) as a1wb,
                tc.tile_pool(name="a1land", bufs=2) as a1land,
                tc.tile_pool(name="a1xcb", bufs=4) as a1xcb,
                tc.tile_pool(name="a1stage", bufs=4) as a1stage,
                tc.tile_pool(name="a1ps", bufs=4, space="PSUM") as a1ps,
            ):
                wk_b = {}
                for i, stream in enumerate(("x", "c")):
                    wk_b[stream] = a1wb.tile([128, NJ, FQK], BF16, tag="wkb",
                                             name=f"wk_{stream}")
                    load_cast_w(f"wk{stream}", wk_b[stream], a1w, nc.vector)
                # wq loads (into outer pool), cast on Pool — needed from A2 on
                for stream in ("x", "c"):
                    load_cast_w(f"wq{stream}", wq_b[stream], a1w, nc.gpsimd)

                for ck in range(NCK):
                    for stream, src_r in (("x", xr), ("c", cr)):
                        xcb = load_chunk(a1land, a1xcb, src_r, ck, f"a1{stream}{ck}")
                        kstage = a1stage.tile([128, 4, CK], BF16, tag="kst",
                                              name=f"kst{stream}{ck}")
                        for fi in range(4):
                            ps = a1ps.tile([128, CK], F32, tag="kps", name="kps")
                            for j in range(NJ):
                                nc.tensor.matmul(
                                    ps,
                                    wk_b[stream][:, j, fi * 128:(fi + 1) * 128],
                                    xcb[:, j, :],
                                    start=(j == 0),
                                    stop=(j == NJ - 1),
                                )
                            nc.scalar.activation(
                                out=kstage[:, fi, :], in_=ps,
                                func=mybir.ActivationFunctionType.Copy,
                            )
                        poff = 0 if stream == "x" else 64
                        for hp in range(2):
                            nc.sync.dma_start(
                                out=k_re[poff:poff + 64, :, hp,
                                         ck * CK:(ck + 1) * CK],
                                in_=kstage[64 * hp:64 * hp + 64, :, :],
                            )

            # ------ phase A2: V projection + slab-0 Q proj + warmup ------
            with (
                tc.tile_pool(name="a2w", bufs=2) as a2w,
                tc.tile_pool(name="a2wb", bufs=2) as a2wb,
                tc.tile_pool(name="a2land", bufs=2) as a2land,
                tc.tile_pool(name="a2xcb", bufs=4) as a2xcb,
                tc.tile_pool(name="a2stage", bufs=4) as a2stage,
                tc.tile_pool(name="a2vps", bufs=4, space="PSUM") as a2vps,
                tc.tile_pool(name="a2qps", bufs=2, space="PSUM") as a2qps,
                tc.tile_pool(name="a2sps", bufs=2, space="PSUM") as a2sps,
            ):
                wv_b = {}
                for stream in ("x", "c"):
                    wv_b[stream] = a2wb.tile([128, NJ, FQK], BF16, tag="wvb",
                                             name=f"wv_{stream}")
                    load_cast_w(f"wv{stream}", wv_b[stream], a2w, nc.vector)

                def qproj_chunk(sp, ck_in_slab, xcb, stream, stagep, qpsp):
                    """One chunk (256 toks) of q projection for slab sp."""
                    qstage = stagep.tile([128, 4, CK], BF16, tag="qst",
                                         name=f"qst{sp}{ck_in_slab}{stream}")
                    for fi in range(4):
                        ps = qpsp.tile([128, CK], F32, tag="qps", name="qps")
                        for j in range(NJ):
                            nc.tensor.matmul(
                                ps,
                                wq_b[stream][:, j, fi * 128:(fi + 1) * 128],
                                xcb[:, j, :],
                                start=(j == 0),
                                stop=(j == NJ - 1),
                            )
                        nc.vector.tensor_copy(qstage[:, fi, :], ps)
                    q_re = q_tiles[sp % 2].rearrange(
                        "p (fi hp) t -> p fi hp t", fi=4)
                    poff = 0 if stream == "x" else 64
                    for hp in range(2):
                        nc.sync.dma_start(
                            out=q_re[poff:poff + 64, :, hp,
                                     ck_in_slab * CK:(ck_in_slab + 1) * CK],
                            in_=qstage[64 * hp:64 * hp + 64, :, :],
                        )

                # exp tiles: ring in outer pool (alive into the attention loop)
                ex_of = {}

                def emit_scores_exp(G, kt, qsrc):
                    sp, h = divmod(G, 8)
                    ps = attn_sps.tile([128, PAN], F32, tag="sps", name="sps")
                    nc.tensor.matmul(
                        ps, k_sb[:, h, kt * 128:(kt + 1) * 128],
                        qsrc[:, h, :], start=True, stop=True,
                    )
                    ex = expool.tile([128, PAN], BF16, tag="ex",
                                     name=f"ex{G}_{kt}")
                    nc.scalar.activation(
                        out=ex, in_=ps,
                        func=mybir.ActivationFunctionType.Exp,
                        scale=float(SCALE),
                    )
                    ex_of[(G, kt)] = ex

                attn_sps = a2sps  # warmup scores use A2 psum ring

                # chunk loads + qproj for slab 0 (chunks 0,1), then vproj all
                vchunks = {}
                for ck in range(2):
                    for stream, src_r in (("x", xr), ("c", cr)):
                        xcb = load_chunk(a2land, a2xcb, src_r, ck,
                                         f"a2{stream}{ck}")
                        vchunks[(ck, stream)] = xcb
                        qproj_chunk(0, ck, xcb, stream, a2stage, a2qps)

                warm = 0
                for ck in range(NCK):
                    for stream in ("x", "c"):
                        if (ck, stream) in vchunks:
                            xcb = vchunks.pop((ck, stream))
                        else:
                            xcb = load_chunk(a2land, a2xcb,
                                             xr if stream == "x" else cr,
                                             ck, f"a2{stream}{ck}")
                        voff = 0 if stream == "x" else 64
                        for tt in range(2):
                            kt = ck * 2 + tt
                            ps = a2vps.tile([128, FQK], F32, tag="vps",
                                            name="vps")
                            for j in range(NJ):
                                nc.tensor.matmul(
                                    ps,
                                    xcb[:, j, tt * 128:(tt + 1) * 128],
                                    wv_b[stream][:, j, :],
                                    start=(j == 0),
                                    stop=(j == NJ - 1),
                                )
                            nc.scalar.activation(
                                out=v_sb[:, kt, :, voff:voff + 64],
                                in_=ps.rearrange("p (h d) -> p h d", h=HL),
                                func=mybir.ActivationFunctionType.Copy,
                            )
                            # warmup: interleave slab-0 head-0 scores
                            if warm < NT:
                                emit_scores_exp(0, warm, q_tiles[0])
                                warm += 1

                # qproj for slab 1 as well (PE work overlapping exp warmup)
                for ck_in_slab in range(2):
                    for stream, src_r in (("x", xr), ("c", cr)):
                        xcb = load_chunk(a2land, a2xcb, src_r, 2 + ck_in_slab,
                                         f"a2q1{stream}{ck_in_slab}")
                        qproj_chunk(1, ck_in_slab, xcb, stream, a2stage,
                                    a2qps)

            # ---------------- pipelined attention main loop ----------------
            import contextlib

            with contextlib.ExitStack() as atctx:
                P = lambda *a, **kw: atctx.enter_context(tc.tile_pool(*a, **kw))
                wpp = P(name="wpp", bufs=1)
                wpl = P(name="wpl", bufs=1)
                atland = P(name="atland", bufs=1)
                atxcb = P(name="atxcb", bufs=4)
                atstage = P(name="atstage", bufs=2)
                tmpp = P(name="tmp", bufs=3)
                accp = P(name="accp", bufs=1)
                invp = P(name="invp", bufs=1)
                aop = P(name="aop", bufs=2)
                y3 = P(name="y3", bufs=1)
                at_sps = P(name="at_sps", bufs=3, space="PSUM")
                at_out = P(name="at_out", bufs=2, space="PSUM")
                at_den = P(name="at_den", bufs=1, space="PSUM")
                at_yps = P(name="at_yps", bufs=1, space="PSUM")
                at_qps = P(name="at_qps", bufs=1, space="PSUM")
                attn_sps = at_sps

                # wp: load f32 in quarters, cast to bf16 on Pool
                wp_b = wpp.tile([128, HL, D], BF16, tag="wp_b")
                for qu in range(4):
                    land = wpl.tile([128, 2, D], F32, tag="wpl", name="wpland")
                    nc.sync.dma_start(
                        out=land,
                        in_=wp_in.rearrange("(j p) f -> p j f", p=128)[
                            :, qu * 2:(qu + 1) * 2, :],
                    )
                    nc.gpsimd.tensor_copy(wp_b[:, qu * 2:(qu + 1) * 2, :],
                                          land)
                bias_b = wpp.tile([128, D], F32, tag="bias_b")
                b_ap = bias_in[:]
                nc.sync.dma_start(
                    out=bias_b,
                    in_=bass.AP(
                        tensor=b_ap.tensor,
                        offset=b_ap.offset,
                        ap=[[0, 128]] + [list(p) for p in b_ap.ap],
                    ),
                )

                ao_tiles = [aop.tile([128, HL, PAN], BF16, tag="ao",
                                     name=f"ao{i}") for i in range(2)]
                p_out_of = {}

                def emit_av(Gp, kt):
                    sp, h = divmod(Gp, 8)
                    if kt == 0:
                        p_out_of[Gp] = at_out.tile([128, PAN], F32, tag="pout",
                                                   name=f"pout{Gp}")
                    nc.tensor.matmul(
                        p_out_of[Gp], v_sb[:, kt, h, :], ex_of[(Gp, kt)],
                        start=(kt == 0), stop=(kt == NT - 1),
                    )

                tree = {}

                def emit_tree(Gp, kt):
                    # after exp(Gp, kt) emitted, emit ready reduction-adds
                    if kt % 2 == 1:
                        i = kt // 2
                        t = tmpp.tile([128, PAN], F32, tag="l1",
                                      name=f"l1_{Gp}_{i}")
                        nc.vector.tensor_add(t, ex_of[(Gp, kt - 1)],
                                             ex_of[(Gp, kt)])
                        tree[(Gp, 1, i)] = t
                    if kt % 4 == 3:
                        i = kt // 4
                        t = tmpp.tile([128, PAN], F32, tag="l2", bufs=2,
                                      name=f"l2_{Gp}_{i}")
                        nc.vector.tensor_add(t, tree.pop((Gp, 1, 2 * i)),
                                             tree.pop((Gp, 1, 2 * i + 1)))
                        tree[(Gp, 2, i)] = t
                    if kt % 8 == 7:
                        i = kt // 8
                        t = tmpp.tile([128, PAN], F32, tag="l3", bufs=2,
                                      name=f"l3_{Gp}_{i}")
                        nc.vector.tensor_add(t, tree.pop((Gp, 2, 2 * i)),
                                             tree.pop((Gp, 2, 2 * i + 1)))
                        tree[(Gp, 3, i)] = t
                    if kt == 15:
                        acc = accp.tile([128, PAN], F32R, tag="acc",
                                        name=f"acc{Gp}")
                        nc.vector.tensor_add(acc, tree.pop((Gp, 3, 0)),
                                             tree.pop((Gp, 3, 1)))
                        tree[(Gp, 4, 0)] = acc

                def finish_block(Gp):
                    """ones-matmul + reciprocal + normalize for block Gp."""
                    sp, h = divmod(Gp, 8)
                    acc = tree.pop((Gp, 4, 0))
                    den = at_den.tile([128, PAN], F32, tag="den", name="den")
                    nc.tensor.matmul(den, ones_r, acc,
                                     start=True, stop=True)
                    inv = invp.tile([128, PAN], F32, tag="inv",
                                    name=f"inv{Gp}")
                    nc.vector.reciprocal_approx_fast(inv, den)
                    nc.vector.tensor_mul(ao_tiles[sp % 2][:, h, :],
                                         p_out_of.pop(Gp), inv)
                    for kt in range(NT):
                        del ex_of[(Gp, kt)]

                def emit_yproj_group(sp, b):
                    """yproj group b (of 8) for slab sp: one psum tile."""
                    tt, do = b // 2, b % 2
                    ao = ao_tiles[sp % 2]
                    ps = at_yps.tile([128, PAN], F32, tag="yps", name="yps")
                    for fi in range(HL):
                        nc.tensor.matmul(
                            ps, ao[:, fi, tt * 128:(tt + 1) * 128],
                            wp_b[:, fi, do * PAN:(do + 1) * PAN],
                            start=(fi == 0), stop=(fi == HL - 1),
                        )
                    yt = y3.tile([128, PAN], F32, tag="yt", name="yt")
                    nc.vector.tensor_add(yt, ps,
                                         bias_b[:, do * PAN:(do + 1) * PAN])
                    nc.gpsimd.dma_start(
                        out=y_part[sp][tt * 128:(tt + 1) * 128,
                                       do * PAN:(do + 1) * PAN],
                        in_=yt,
                    )
                    if b == 7:
                        nc.gpsimd.collective_compute(
                            "AllReduce",
                            mybir.AluOpType.add,
                            replica_groups=[[0, 1], [2, 3], [4, 5], [6, 7]],
                            ins=[y_part[sp][:]],
                            outs=[y_red[sp][:]],
                        )
                        nc.gpsimd.dma_start(
                            out=y_out[sp * PAN:(sp + 1) * PAN, :],
                            in_=y_red[sp][:],
                        )

                # qproj chunk scheduling state for slabs 1..3
                at_chunks = {}

                def load_at_chunk(sp, ck_in_slab, stream):
                    src_r = xr if stream == "x" else cr
                    ck = sp * 2 + ck_in_slab
                    land = atland.tile([128, NJ, CK], F32, tag="land",
                                       name=f"atl{sp}{ck_in_slab}{stream}")
                    nc.sync.dma_start(
                        out=land, in_=src_r[:, :, ck * CK:(ck + 1) * CK])
                    xcb = atxcb.tile([128, NJ, CK], BF16, tag="xcb",
                                     name=f"atb{sp}{ck_in_slab}{stream}")
                    nc.gpsimd.tensor_copy(xcb, land)
                    at_chunks[(sp, ck_in_slab, stream)] = xcb

                def emit_qproj_part(sp_next, part):
                    """part 0..7: (chunk, stream, fi-pair) slices of qproj."""
                    ck_in_slab, stream = part // 4, ("x", "c")[(part // 2) % 2]
                    # two fi per part
                    xcb = at_chunks[(sp_next, ck_in_slab, stream)]
                    qstage_key = (sp_next, ck_in_slab, stream)
                    if part % 2 == 0:
                        qst = atstage.tile([128, 4, CK], BF16, tag="qst",
                                           name=f"qs{sp_next}{ck_in_slab}{stream}")
                        at_qstage[qstage_key] = qst
                    qst = at_qstage[qstage_key]
                    for fi in range(2 * (part % 2), 2 * (part % 2) + 2):
                        ps = at_qps.tile([128, CK], F32, tag="qps", name="qps")
                        for j in range(NJ):
                            nc.tensor.matmul(
                                ps,
                                wq_b[stream][:, j, fi * 128:(fi + 1) * 128],
                                xcb[:, j, :],
                                start=(j == 0), stop=(j == NJ - 1),
                            )
                        nc.vector.tensor_copy(qst[:, fi, :], ps)
                    if part % 2 == 1:
                        q_re = q_tiles[sp_next % 2].rearrange(
                            "p (fi hp) t -> p fi hp t", fi=4)
                        poff = 0 if stream == "x" else 64
                        for hp in range(2):
                            nc.sync.dma_start(
                                out=q_re[poff:poff + 64, :, hp,
                                         ck_in_slab * CK:(ck_in_slab + 1) * CK],
                                in_=qst[64 * hp:64 * hp + 64, :, :],
                            )

                at_qstage = {}

                # main loop: G = block whose scores are emitted; G-1 gets AV.
                for G in range(1, 33):
                    sp_G, h_G = divmod(G, 8) if G < 32 else (None, None)
                    Gp = G - 1
                    sp_P, h_P = divmod(Gp, 8)
                    for kt in range(NT):
                        emit_av(Gp, kt)
                        if G < 32:
                            emit_scores_exp(G, kt, q_tiles[sp_G % 2])
                        emit_tree(Gp, kt)
                        # fillers
                        # qproj for slabs 2,3: chunk loads fire in the last
                        # block of slab T-2 (G=7 -> slab 2, G=15 -> slab 3);
                        # parts run during slab T-1's blocks at kt==5.
                        if G in (7, 15) and kt in (2, 6, 10, 14):
                            load_at_chunk(G // 8 + 2, int(kt > 6),
                                          "x" if kt in (2, 10) else "c")
                        if G < 32 and 1 <= sp_G <= 2 and kt == 5:
                            emit_qproj_part(sp_G + 1, h_G)
                        if kt == 9 and sp_P >= 1:
                            # yproj for slab sp_P-1, group = h_P
                            emit_yproj_group(sp_P - 1, h_P)
                    finish_block(Gp)
                # epilogue: yproj for slab 3
                for b in range(8):
                    emit_yproj_group(3, b)

    nc.finalize()
    return nc


_NC = {}


def _get_nc(reps=1):
    global _NC
    if _NC is None:
        _NC = {}
    if reps not in _NC:
        _NC[reps] = _build_nc()
    return _NC[reps]


def _shard_inputs(inputs):
    x = np.ascontiguousarray(inputs["x"], dtype=np.float32)
    c = np.ascontiguousarray(inputs["c"], dtype=np.float32)
    wq_x, wk_x, wv_x = inputs["Wq_x"], inputs["Wk_x"], inputs["Wv_x"]
    wq_c, wk_c, wv_c = inputs["Wq_c"], inputs["Wk_c"], inputs["Wv_c"]
    w_proj, b_proj = inputs["W_proj"], inputs["b_proj"]

    in_maps = []
    for core in range(N_CORES):
        b, g = core // 2, core % 2
        fs = slice(g * FQK, (g + 1) * FQK)
        m = {
            "xt": np.ascontiguousarray(x[b].T),
            "ct": np.ascontiguousarray(c[b].T),
            "wqx": np.ascontiguousarray(wq_x[:, fs], dtype=np.float32),
            "wqc": np.ascontiguousarray(wq_c[:, fs], dtype=np.float32),
            "wkx": np.ascontiguousarray(wk_x[:, fs], dtype=np.float32),
            "wkc": np.ascontiguousarray(wk_c[:, fs], dtype=np.float32),
            "wvx": np.ascontiguousarray(wv_x[:, fs], dtype=np.float32),
            "wvc": np.ascontiguousarray(wv_c[:, fs], dtype=np.float32),
            "wp": np.ascontiguousarray(
                w_proj[g * HL * 2 * DV:(g + 1) * HL * 2 * DV, :],
                dtype=np.float32),
            "bias": (
                np.ascontiguousarray(b_proj, dtype=np.float32)
                if g == 0
                else np.zeros((D,), np.float32)
            ),
        }
        in_maps.append(m)
    return in_maps


def kernel(**inputs) -> np.ndarray:
    from concourse.bass_utils import run_bass_kernel_spmd

    nc = _get_nc()
    in_maps = _shard_inputs(inputs)
    res = run_bass_kernel_spmd(nc, in_maps, list(range(N_CORES)))
    y = np.stack([res.results[2 * b]["y"] for b in range(B)], axis=0)
    return y.astype(np.float32)
